# revision 1
# baseline (speedup 1.0000x reference)
"""TRN2 Bass kernel for nn_Encoder (two-phase LSTM over huge batch).

Self-contained: takes the FULL unsharded inputs, shards the batch across
8 NeuronCores (pure data parallel), runs a Bass/Tile kernel per core via
run_bass_kernel_spmd, and reassembles the full outputs.

Device layout (per core, batch B_c = 65536):
  - batch split into 8 passes of 16*512; slice s=0..15 covers 512 columns
    of a pass; SBUF partition p = 8*s + r  <->  (slice s, feature r).
  - one fp16 matmul per gate bank per step: M=128, K=128, block-diagonal
    lhsT (16 8x8 blocks) reads the whole h/x tile in place and produces
    that bank for all 16 slices at once (fp32 matmul would be lowered to
    two LO/HI PE passes - 2x the PE time - so weights/data are fp16 while
    PSUM accumulates fp32).
  - x-tiles pack 3 timesteps (row 2*tau+k = x[t0+tau][k]) plus a ones row
    that carries the fused bias; the host bakes this layout (fp16) so
    every DMA is a contiguous [128, 512] transfer.
  - PSUM gate banks [F, I, O, G] as [128, 4, 512] tiles from a rotating
    2-slot pool (8 banks total = pipeline depth 2).
  - ACT: sigmoid over F/I/O in one instr, tanh over G, tanh(c) - the ACT
    engine is the bottleneck (~40 transcendentals per element per step).
  - DVE (all fp16, 2x mode): u=F*c, v=I*G, c=u+v, h=O*tanh(c).
  - input embedding + biases are folded into the lhsT weights on the host
    (gates = x @ (W_ih W_in).T + h @ W_hh.T + (W_ih b_in + b_ih + b_hh)).
  - 8 passes run as 8 interleaved chains, steps emitted round-robin so
    PSUM slots rotate across chains (pass-sequential emission serializes
    the whole pipeline through the slot WAR chain).
"""

import os
import sys

for _p in ("/opt/trn_rl_repo", "/root/.axon_site/_ro/trn_rl_repo"):
    if os.path.isdir(_p) and _p not in sys.path:
        sys.path.insert(0, _p)
        break

import numpy as np

import concourse.bacc as bacc
import concourse.mybir as mybir
import concourse.tile as tile
from concourse import bass_utils

F32 = mybir.dt.float32
F16 = mybir.dt.float16
AF = mybir.ActivationFunctionType

B = 524288
N_CORES = 8
B_C = B // N_CORES
N = 512
SLICES = 16
PASS = SLICES * N
N_PASS = B_C // PASS
T_OBS, T_PRE, IN, H = 8, 12, 2, 8
XPACK = 3
N_CHUNK_OBS = (T_OBS + XPACK - 1) // XPACK
N_CHUNK_PRE = (T_PRE + XPACK - 1) // XPACK
N_CHAINS = 8
# bank order: F, I, O, G (sigmoid banks contiguous, tanh last); pytorch
# gate order in the weight rows is i, f, g, o.
BANK_GATE = [1, 0, 3, 2]


# ---------------------------------------------------------------- host prep

def _make_weights(W_in, b_in, W_ih, W_hh, b_ih, b_hh):
    """lhsT arrays: w_gx [XPACK, 128, 4, 128] (tau,p,bank,m), w_gh [128,4,128].

    Block-diagonal over the 16 slices: one M=128, K=128 matmul per gate bank
    computes that bank for all 16 slices at once.
    """
    Wx = (W_ih @ W_in).astype(np.float32)
    bias = (W_ih @ b_in + b_ih + b_hh).astype(np.float32)
    w_gx = np.zeros((XPACK, 128, 4, 128), np.float32)
    w_gh = np.zeros((128, 4, 128), np.float32)
    for b in range(4):
        g = BANK_GATE[b]
        for s in range(16):
            for r in range(H):
                col = 8 * s + r
                for tau in range(XPACK):
                    for k in range(IN):
                        w_gx[tau, 8 * s + 2 * tau + k, b, col] = Wx[g * H + r, k]
                    w_gx[tau, 8 * s + 6, b, col] = bias[g * H + r]
                w_gh[8 * s: 8 * s + H, b, col] = W_hh[g * H + r, :]
    return w_gx.astype(np.float16), w_gh.astype(np.float16)


def _shuffle_state(aT):
    """[8, B_c] -> [N_PASS, 128, N] device layout (p, 8s+r, n)."""
    return np.ascontiguousarray(
        aT.reshape(H, N_PASS, SLICES, N).transpose(1, 2, 0, 3).reshape(
            N_PASS, 128, N).astype(np.float16))


def _unshuffle_state(dev):
    """[N_PASS, 128, N] -> [8, B_c]."""
    return dev.reshape(N_PASS, SLICES, H, N).transpose(2, 0, 1, 3).reshape(
        H, B_C)


def _pack_x(x):
    """[T, 2, B_c] -> [n_chunk, N_PASS, 128, N]: 3 steps + ones row baked."""
    T = x.shape[0]
    n_chunk = (T + XPACK - 1) // XPACK
    out = np.zeros((n_chunk, N_PASS, SLICES, 8, N), np.float32)
    out[:, :, :, 6, :] = 1.0
    for tau in range(XPACK):
        for k in range(IN):
            for t3 in range(n_chunk):
                t = t3 * XPACK + tau
                if t < T:
                    out[t3, :, :, 2 * tau + k, :] = x[t, k].reshape(
                        N_PASS, SLICES, N)
    return np.ascontiguousarray(
        out.reshape(n_chunk, N_PASS, 128, N).astype(np.float16))


def _prep_core_inputs(inputs, lo, hi, weights):
    g = lambda k: np.asarray(inputs[k], np.float32)
    d = {}
    d["x_obs"] = _pack_x(
        np.ascontiguousarray(g("obs_traj_rel")[:, lo:hi, :].transpose(0, 2, 1)))
    d["x_pre"] = _pack_x(
        np.ascontiguousarray(g("pre_traj_rel")[:, lo:hi, :].transpose(0, 2, 1)))
    d["hT0"] = _shuffle_state(np.ascontiguousarray(g("h0")[lo:hi].T))
    d["cT0"] = _shuffle_state(np.ascontiguousarray(g("c0")[lo:hi].T))
    d["cT0_pre"] = _shuffle_state(np.ascontiguousarray(g("c0_pre")[lo:hi].T))
    d.update(weights)
    return d


# ------------------------------------------------------------- device build

def _build_kernel(tc, outs, ins):
    nc = tc.nc
    state = tc.alloc_tile_pool(name="state", bufs=1)
    psump = tc.alloc_tile_pool(name="psum", bufs=2, space="PSUM")

    wsb = {}
    for key in ("w_gx_obs", "w_gx_pre"):
        w = state.tile([128, XPACK, 4, 128], F16, name=key + "_sb", tag=key)
        nc.sync.dma_start(w, ins[key].rearrange("t p b m -> p t b m"))
        wsb[key] = w
    for key in ("w_gh_obs", "w_gh_pre"):
        w = state.tile([128, 4, 128], F16, name=key + "_sb", tag=key)
        nc.sync.dma_start(w, ins[key])
        wsb[key] = w

    chains = []
    for ci in range(N_CHAINS):
        ch = {}
        for nm in ("h", "c", "p1", "p2"):
            ch[nm] = state.tile([128, N], F16, name=f"{nm}_{ci}",
                                tag=f"{nm}_{ci}")
        ch["T"] = state.tile([128, 4, N], F16, name=f"T_{ci}", tag=f"T_{ci}")
        ch["xs"] = [
            state.tile([128, N], F16, name=f"x_{ci}_{xi}", tag=f"x_{ci}_{xi}")
            for xi in range(2)
        ]
        chains.append(ch)

    def step(ch, wgx, wgh, tau, xt):
        h, c, p1, p2, T = (ch[k] for k in ("h", "c", "p1", "p2", "T"))
        ps = psump.tile([128, 4, 512], F32, name="ps", tag="ps")
        for b in range(4):
            out = ps[:, b, :N]
            nc.tensor.matmul(out, wgx[:, tau, b, :], xt,
                             start=True, stop=False)
            nc.tensor.matmul(out, wgh[:, b, :], h,
                             start=False, stop=True)
        nc.scalar.activation(T[:, 0:3, :], ps[:, 0:3, :N], AF.Sigmoid)
        nc.scalar.activation(T[:, 3, :], ps[:, 3, :N], AF.Tanh)
        Fg, Ig, Og, Gg = (T[:, b, :] for b in range(4))
        nc.vector.tensor_mul(p1, Fg, c)          # u = F*c
        nc.vector.tensor_mul(p2, Ig, Gg)         # v = I*G
        nc.vector.tensor_add(c, p1, p2)          # c = u+v
        nc.scalar.activation(p1, c, AF.Tanh)     # tc = tanh(c)
        nc.vector.tensor_mul(h, Og, p1)          # h = O*tc

    assert N_PASS % N_CHAINS == 0
    for g0 in range(0, N_PASS, N_CHAINS):
        group = [(chains[ci], g0 + ci) for ci in range(N_CHAINS)]
        for ch, p in group:
            nc.sync.dma_start(ch["h"], ins["hT0"][p])
            nc.sync.dma_start(ch["c"], ins["cT0"][p])
        for t in range(T_OBS + T_PRE):
            for ch, p in group:
                if t < T_OBS:
                    which, tt = "obs", t
                else:
                    which, tt = "pre", t - T_OBS
                if t == T_OBS:
                    nc.sync.dma_start(outs["hT_obs"][p], ch["h"])
                    nc.sync.dma_start(ch["c"], ins["cT0_pre"][p])
                wgx, wgh = wsb[f"w_gx_{which}"], wsb[f"w_gh_{which}"]
                t3, tau = divmod(tt, XPACK)
                if tau == 0:
                    nc.sync.dma_start(ch["xs"][t3 % 2],
                                      ins[f"x_{which}"][t3, p])
                step(ch, wgx, wgh, tau, ch["xs"][t3 % 2])
        for ch, p in group:
            nc.sync.dma_start(outs["hT_pre"][p], ch["h"])

    state.release()
    psump.release()


_CACHED = {}


def _get_program():
    if "nc" in _CACHED:
        return _CACHED["nc"], _CACHED["names"]
    nc = bacc.Bacc("TRN2", target_bir_lowering=False, debug=False,
                   enable_asserts=False, num_devices=N_CORES)
    in_shapes = {
        "x_obs": (N_CHUNK_OBS, N_PASS, 128, N),
        "x_pre": (N_CHUNK_PRE, N_PASS, 128, N),
        "hT0": (N_PASS, 128, N),
        "cT0": (N_PASS, 128, N),
        "cT0_pre": (N_PASS, 128, N),
        "w_gx_obs": (XPACK, 128, 4, 128),
        "w_gh_obs": (128, 4, 128),
        "w_gx_pre": (XPACK, 128, 4, 128),
        "w_gh_pre": (128, 4, 128),
    }
    ins = {
        k: nc.dram_tensor(k, list(s), F16, kind="ExternalInput").ap()
        for k, s in in_shapes.items()
    }
    outs = {
        k: nc.dram_tensor(k, [N_PASS, 128, N], F16, kind="ExternalOutput").ap()
        for k in ("hT_obs", "hT_pre")
    }
    with tile.TileContext(nc) as tc:
        _build_kernel(tc, outs, ins)
    nc.compile()
    _CACHED["nc"] = nc
    _CACHED["names"] = list(in_shapes)
    return nc, _CACHED["names"]


def run(inputs, trace=False, trace_kwargs=None):
    """Run the kernel on 8 cores; returns ((c_out, x_out), BassKernelResults)."""
    nc, _ = _get_program()
    g = lambda k: np.asarray(inputs[k], np.float32)
    wgx_o, wgh_o = _make_weights(g("W_in"), g("b_in"), g("W_ih_obs"),
                                 g("W_hh_obs"), g("b_ih_obs"), g("b_hh_obs"))
    wgx_p, wgh_p = _make_weights(g("W_in"), g("b_in"), g("W_ih_pre"),
                                 g("W_hh_pre"), g("b_ih_pre"), g("b_hh_pre"))
    weights = {"w_gx_obs": wgx_o, "w_gh_obs": wgh_o,
               "w_gx_pre": wgx_p, "w_gh_pre": wgh_p}
    in_maps = [
        _prep_core_inputs(inputs, c * B_C, (c + 1) * B_C, weights)
        for c in range(N_CORES)
    ]
    res = bass_utils.run_bass_kernel_spmd(
        nc, in_maps, core_ids=list(range(N_CORES)), trace=trace,
        **(trace_kwargs or {}))
    hT_obs = np.concatenate(
        [_unshuffle_state(res.results[c]["hT_obs"]) for c in range(N_CORES)],
        axis=1)
    hT_pre = np.concatenate(
        [_unshuffle_state(res.results[c]["hT_pre"]) for c in range(N_CORES)],
        axis=1)
    c_out = hT_obs.reshape(B, H).astype(np.float32)
    x_out = hT_pre.reshape(B, H).astype(np.float32)
    return (c_out, x_out), res


def kernel(**inputs):
    (c_out, x_out), _ = run(inputs)
    return c_out, x_out



# revision 3
# speedup vs baseline: 1.4067x; 1.4067x over previous
"""TRN2 Bass kernel for nn_Encoder (two-phase LSTM over huge batch).

Self-contained: takes the FULL unsharded inputs, shards the batch across
8 NeuronCores (pure data parallel), runs a Bass/Tile kernel per core via
run_bass_kernel_spmd, and reassembles the full outputs.

Device layout (per core, batch B_c = 65536):
  - batch split into 8 passes of 16*512; slice s=0..15 covers 512 columns
    of a pass; SBUF partition p = 8*s + r  <->  (slice s, feature r).
  - one fp16 matmul per gate bank per step: M=128, K=128, block-diagonal
    lhsT (16 8x8 blocks) reads the whole h/x tile in place and produces
    that bank for all 16 slices at once.
  - x-tiles pack 3 timesteps (row 2*tau+k = x[t0+tau][k]) plus a ones row
    that carries the fused bias; the host bakes this layout (fp16) so
    every DMA is a contiguous [128, 512] transfer.
  - PSUM gate banks [F, I, O, G] as one [128, 4, 512] tile from a rotating
    2-slot pool.
  - ACT is the bottleneck engine (~40 transcendentals per element per
    step at 1 elem/cycle/lane + ~352 cy fixed cost per ACTIVATE), so the
    schedule minimizes ACT instructions:
      * G-gate weights/bias are pre-doubled on the host, so
        tanh(g) = 2*sigmoid(2g) - 1 and ALL FOUR banks go through a
        single Sigmoid ACTIVATE per step (PSUM src, 2048 elems).
      * tanh(c) is batched across a 4-chain group into one ACTIVATE
        (chains keep c in one contiguous group tile).
      * group tanh instructions are emitted offset by one chain so ACT
        never waits on the DVE cell-update chain.
  - DVE (all fp16): tg=2*S_G-1 (tensor_scalar, 4x mode), u=F*c, v=I*tg,
    c=u+v, h=O*tanh_c (tensor_tensor, 2x mode).
  - input embedding + biases are folded into the lhsT weights on the host
    (gates = x @ (W_ih W_in).T + h @ W_hh.T + (W_ih b_in + b_ih + b_hh)).
"""

import os
import sys

for _p in ("/opt/trn_rl_repo", "/root/.axon_site/_ro/trn_rl_repo"):
    if os.path.isdir(_p) and _p not in sys.path:
        sys.path.insert(0, _p)
        break

import numpy as np

import concourse.bacc as bacc
import concourse.mybir as mybir
import concourse.tile as tile
from concourse import bass_utils

F32 = mybir.dt.float32
F16 = mybir.dt.float16
AF = mybir.ActivationFunctionType
ALU = mybir.AluOpType

B = 524288
N_CORES = 8
B_C = B // N_CORES
N = 512
SLICES = 16
PASS = SLICES * N
N_PASS = B_C // PASS
T_OBS, T_PRE, IN, H = 8, 12, 2, 8
XPACK = 3
N_CHUNK_OBS = (T_OBS + XPACK - 1) // XPACK
N_CHUNK_PRE = (T_PRE + XPACK - 1) // XPACK
N_CHAINS = 8
GRP = 4  # chains per tanh(c) batch group
# bank order: F, I, O, G (sigmoid banks contiguous, tanh last); pytorch
# gate order in the weight rows is i, f, g, o.
BANK_GATE = [1, 0, 3, 2]


# ---------------------------------------------------------------- host prep

def _make_weights(W_in, b_in, W_ih, W_hh, b_ih, b_hh):
    """lhsT arrays: w_gx [XPACK, 128, 4, 128] (tau,p,bank,m), w_gh [128,4,128].

    Block-diagonal over the 16 slices: one M=128, K=128 matmul per gate bank
    computes that bank for all 16 slices at once.  Bank 3 (the candidate
    gate g) is pre-scaled by 2 so tanh(g) = 2*sigmoid(2g) - 1 on device.
    """
    Wx = (W_ih @ W_in).astype(np.float32)
    bias = (W_ih @ b_in + b_ih + b_hh).astype(np.float32)
    w_gx = np.zeros((XPACK, 128, 4, 128), np.float32)
    w_gh = np.zeros((128, 4, 128), np.float32)
    for b in range(4):
        g = BANK_GATE[b]
        sc = 2.0 if b == 3 else 1.0
        for s in range(16):
            for r in range(H):
                col = 8 * s + r
                for tau in range(XPACK):
                    for k in range(IN):
                        w_gx[tau, 8 * s + 2 * tau + k, b, col] = sc * Wx[g * H + r, k]
                    w_gx[tau, 8 * s + 6, b, col] = sc * bias[g * H + r]
                w_gh[8 * s: 8 * s + H, b, col] = sc * W_hh[g * H + r, :]
    return w_gx.astype(np.float16), w_gh.astype(np.float16)


def _shuffle_state(aT):
    """[8, B_c] -> [N_PASS, 128, N] device layout (p, 8s+r, n)."""
    return np.ascontiguousarray(
        aT.reshape(H, N_PASS, SLICES, N).transpose(1, 2, 0, 3).reshape(
            N_PASS, 128, N).astype(np.float16))


def _unshuffle_state(dev):
    """[N_PASS, 128, N] -> [8, B_c]."""
    return dev.reshape(N_PASS, SLICES, H, N).transpose(2, 0, 1, 3).reshape(
        H, B_C)


def _pack_x(x):
    """[T, 2, B_c] -> [n_chunk, N_PASS, 128, N]: 3 steps + ones row baked."""
    T = x.shape[0]
    n_chunk = (T + XPACK - 1) // XPACK
    out = np.zeros((n_chunk, N_PASS, SLICES, 8, N), np.float32)
    out[:, :, :, 6, :] = 1.0
    for tau in range(XPACK):
        for k in range(IN):
            for t3 in range(n_chunk):
                t = t3 * XPACK + tau
                if t < T:
                    out[t3, :, :, 2 * tau + k, :] = x[t, k].reshape(
                        N_PASS, SLICES, N)
    return np.ascontiguousarray(
        out.reshape(n_chunk, N_PASS, 128, N).astype(np.float16))


def _prep_core_inputs(inputs, lo, hi, weights):
    g = lambda k: np.asarray(inputs[k], np.float32)
    d = {}
    d["x_obs"] = _pack_x(
        np.ascontiguousarray(g("obs_traj_rel")[:, lo:hi, :].transpose(0, 2, 1)))
    d["x_pre"] = _pack_x(
        np.ascontiguousarray(g("pre_traj_rel")[:, lo:hi, :].transpose(0, 2, 1)))
    d["hT0"] = _shuffle_state(np.ascontiguousarray(g("h0")[lo:hi].T))
    d["cT0"] = _shuffle_state(np.ascontiguousarray(g("c0")[lo:hi].T))
    d["cT0_pre"] = _shuffle_state(np.ascontiguousarray(g("c0_pre")[lo:hi].T))
    d.update(weights)
    return d


# ------------------------------------------------------------- device build

def _build_kernel(tc, outs, ins):
    nc = tc.nc
    state = tc.alloc_tile_pool(name="state", bufs=1)
    psump = tc.alloc_tile_pool(name="psum", bufs=2, space="PSUM")

    wsb = {}
    for key in ("w_gx_obs", "w_gx_pre"):
        w = state.tile([128, XPACK, 4, 128], F16, name=key + "_sb", tag=key)
        nc.sync.dma_start(w, ins[key].rearrange("t p b m -> p t b m"))
        wsb[key] = w
    for key in ("w_gh_obs", "w_gh_pre"):
        w = state.tile([128, 4, 128], F16, name=key + "_sb", tag=key)
        nc.sync.dma_start(w, ins[key])
        wsb[key] = w

    # per-group c and tanh(c) tiles: chain ci -> group ci // GRP, slot ci % GRP
    n_grp = N_CHAINS // GRP
    cgrp = [state.tile([128, GRP, N], F16, name=f"cg_{g}", tag=f"cg_{g}")
            for g in range(n_grp)]
    tgrp = [state.tile([128, GRP, N], F16, name=f"tg_{g}", tag=f"tg_{g}")
            for g in range(n_grp)]

    chains = []
    for ci in range(N_CHAINS):
        ch = {}
        for nm in ("h", "gbar", "u", "v"):
            ch[nm] = state.tile([128, N], F16, name=f"{nm}_{ci}",
                                tag=f"{nm}_{ci}")
        ch["T"] = state.tile([128, 4, N], F16, name=f"T_{ci}", tag=f"T_{ci}")
        ch["xs"] = [
            state.tile([128, N], F16, name=f"x_{ci}_{xi}", tag=f"x_{ci}_{xi}")
            for xi in range(2)
        ]
        ch["c"] = cgrp[ci // GRP][:, ci % GRP, :]
        ch["tc"] = tgrp[ci // GRP][:, ci % GRP, :]
        chains.append(ch)

    def mm_block(ch, wgx, wgh, tau, xt):
        ps = psump.tile([128, 4, 512], F32, name="ps", tag="ps")
        for b in range(4):
            out = ps[:, b, :N]
            nc.tensor.matmul(out, wgx[:, tau, b, :], xt,
                             start=True, stop=False)
            nc.tensor.matmul(out, wgh[:, b, :], ch["h"],
                             start=False, stop=True)
        # single Sigmoid over all 4 banks (G pre-scaled by 2 in weights)
        nc.scalar.activation(ch["T"][:, :, :], ps[:, :, :], AF.Sigmoid)
        T = ch["T"]
        # tg = 2*S_G - 1 == tanh(g)   (tensor_scalar, 4x mode)
        nc.vector.tensor_scalar(ch["gbar"], T[:, 3, :], 2.0, -1.0,
                                ALU.mult, ALU.add)
        nc.vector.tensor_mul(ch["u"], T[:, 0, :], ch["c"])   # F*c
        nc.vector.tensor_mul(ch["v"], T[:, 1, :], ch["gbar"])  # I*tanh(g)
        nc.vector.tensor_add(ch["c"], ch["u"], ch["v"])      # c' = u+v

    def group_tanh(g):
        # one ACTIVATE for the whole group's c, then h = O * tanh(c)
        nc.scalar.activation(tgrp[g][:, :, :], cgrp[g][:, :, :], AF.Tanh)
        for ci in range(g * GRP, (g + 1) * GRP):
            ch = chains[ci]
            nc.vector.tensor_mul(ch["h"], ch["T"][:, 2, :], ch["tc"])

    assert N_PASS == N_CHAINS
    for t in range(T_OBS + T_PRE):
        if t < T_OBS:
            which, tt = "obs", t
        else:
            which, tt = "pre", t - T_OBS
        wgx, wgh = wsb[f"w_gx_{which}"], wsb[f"w_gh_{which}"]
        t3, tau = divmod(tt, XPACK)
        for ci in range(N_CHAINS):
            if ci == 1 and t > 0:
                group_tanh(1)  # tanh(c) for chains 4-7 of step t-1
            ch, p = chains[ci], ci
            if t == 0:
                nc.sync.dma_start(ch["h"], ins["hT0"][p])
                nc.sync.dma_start(ch["c"], ins["cT0"][p])
            if t == T_OBS:
                nc.sync.dma_start(outs["hT_obs"][p], ch["h"])
                nc.sync.dma_start(ch["c"], ins["cT0_pre"][p])
            if tau == 0:
                nc.sync.dma_start(ch["xs"][t3 % 2],
                                  ins[f"x_{which}"][t3, p])
            mm_block(ch, wgx, wgh, tau, ch["xs"][t3 % 2])
            if ci == GRP:
                group_tanh(0)  # tanh(c) for chains 0-3 of step t
    group_tanh(1)  # last step, chains 4-7
    for ci in range(N_CHAINS):
        nc.sync.dma_start(outs["hT_pre"][ci], chains[ci]["h"])

    state.release()
    psump.release()


_CACHED = {}


def _get_program():
    if "nc" in _CACHED:
        return _CACHED["nc"], _CACHED["names"]
    nc = bacc.Bacc("TRN2", target_bir_lowering=False, debug=False,
                   enable_asserts=False, num_devices=N_CORES)
    in_shapes = {
        "x_obs": (N_CHUNK_OBS, N_PASS, 128, N),
        "x_pre": (N_CHUNK_PRE, N_PASS, 128, N),
        "hT0": (N_PASS, 128, N),
        "cT0": (N_PASS, 128, N),
        "cT0_pre": (N_PASS, 128, N),
        "w_gx_obs": (XPACK, 128, 4, 128),
        "w_gh_obs": (128, 4, 128),
        "w_gx_pre": (XPACK, 128, 4, 128),
        "w_gh_pre": (128, 4, 128),
    }
    ins = {
        k: nc.dram_tensor(k, list(s), F16, kind="ExternalInput").ap()
        for k, s in in_shapes.items()
    }
    outs = {
        k: nc.dram_tensor(k, [N_PASS, 128, N], F16, kind="ExternalOutput").ap()
        for k in ("hT_obs", "hT_pre")
    }
    with tile.TileContext(nc) as tc:
        _build_kernel(tc, outs, ins)
    nc.compile()
    _CACHED["nc"] = nc
    _CACHED["names"] = list(in_shapes)
    return nc, _CACHED["names"]


def run(inputs, trace=False, trace_kwargs=None):
    """Run the kernel on 8 cores; returns ((c_out, x_out), BassKernelResults)."""
    nc, _ = _get_program()
    g = lambda k: np.asarray(inputs[k], np.float32)
    wgx_o, wgh_o = _make_weights(g("W_in"), g("b_in"), g("W_ih_obs"),
                                 g("W_hh_obs"), g("b_ih_obs"), g("b_hh_obs"))
    wgx_p, wgh_p = _make_weights(g("W_in"), g("b_in"), g("W_ih_pre"),
                                 g("W_hh_pre"), g("b_ih_pre"), g("b_hh_pre"))
    weights = {"w_gx_obs": wgx_o, "w_gh_obs": wgh_o,
               "w_gx_pre": wgx_p, "w_gh_pre": wgh_p}
    in_maps = [
        _prep_core_inputs(inputs, c * B_C, (c + 1) * B_C, weights)
        for c in range(N_CORES)
    ]
    res = bass_utils.run_bass_kernel_spmd(
        nc, in_maps, core_ids=list(range(N_CORES)), trace=trace,
        **(trace_kwargs or {}))
    hT_obs = np.concatenate(
        [_unshuffle_state(res.results[c]["hT_obs"]) for c in range(N_CORES)],
        axis=1)
    hT_pre = np.concatenate(
        [_unshuffle_state(res.results[c]["hT_pre"]) for c in range(N_CORES)],
        axis=1)
    c_out = hT_obs.reshape(B, H).astype(np.float32)
    x_out = hT_pre.reshape(B, H).astype(np.float32)
    return (c_out, x_out), res


def kernel(**inputs):
    (c_out, x_out), _ = run(inputs)
    return c_out, x_out
